# revision 1
# baseline (speedup 1.0000x reference)
"""Trainium2 Bass kernel for nn_DE_NN_67912022884544 (dense_mlp).

Each population l applies a tiny 1->4->8->4->1 ReLU MLP to a scalar input,
pointwise over a 400k-sample batch.  A scalar->scalar 4-layer ReLU MLP is
exactly a piecewise-linear function of its input, so per population the whole
network collapses (exactly, in real arithmetic) to

    out(x) = A*x + B + sum_k sign_k * |d_k| * relu(x - t_k)

with only ~4-26 knees t_k (computed host-side from the tiny weights; knees
outside [-R, R] fold exactly into A, B for inputs bounded by R — |x| <= 12 is
impossible for randn data).  On device, each knee is one ScalarE activation
pass  relu(scale*x + bias)  plus one VectorE accumulate (add/subtract); the
sample dim rides the 128 SBUF partitions and the free dim.

Sharding: batch is split across the 8 cores (identical SPMD program).  Within
a core, populations are packed 4 per tile (32 sample-lanes each) so one
activation instruction serves 4 populations via per-partition scale/bias
operands; quads are grouped to minimize the total padded slot count.
"""

import os

import numpy as np

NP = 44
B = 400000
NCORES = 8
LANES = 32              # sample lanes per population within a 128-partition tile
PPT = 4                 # populations per tile
NQ = NP // PPT          # 11 quads
SHARD = 50048           # per-core samples per population (128*391; 8*SHARD >= B)
FREE = SHARD // LANES   # 1564
RFOLD = 12.0            # |x| bound used to fold out-of-range knees

LAST_EXEC_NS = None
LAST_RESULTS = None

_PROGRAM_CACHE = {}


# ---------------------------------------------------------------------------
# Host-side exact PWL decomposition (float64, tiny weights only)
# ---------------------------------------------------------------------------

class _PWL:
    """f(x) = a0*x + b0 + sum d*relu(x - t) over knees [(t, d)]."""

    __slots__ = ("a0", "b0", "knees")

    def __init__(self, a0, b0, knees):
        self.a0 = float(a0)
        self.b0 = float(b0)
        self.knees = sorted(knees)

    def segments(self):
        ts = [t for t, _ in self.knees]
        a, b = self.a0, self.b0
        segs = [(a, b)]
        for t, d in self.knees:
            a += d
            b -= d * t
            segs.append((a, b))
        return [-np.inf] + ts + [np.inf], segs

    def __call__(self, x):
        y = self.a0 * x + self.b0
        for t, d in self.knees:
            y += d * max(x - t, 0.0)
        return y


def _lincomb(fs, ws, bias):
    a0 = sum(w * f.a0 for w, f in zip(ws, fs))
    b0 = sum(w * f.b0 for w, f in zip(ws, fs)) + float(bias)
    kn = {}
    for w, f in zip(ws, fs):
        for t, d in f.knees:
            kn[t] = kn.get(t, 0.0) + w * d
    return _PWL(a0, b0, [(t, d) for t, d in kn.items() if d != 0.0])


def _relu_pwl(f):
    bounds, segs = f.segments()
    kn = {}
    for i, (a, b) in enumerate(segs):
        lo, hi = bounds[i], bounds[i + 1]
        if a != 0.0:
            z = -b / a
            if lo < z < hi:
                kn[z] = kn.get(z, 0.0) + abs(a)
    for t, d in f.knees:
        if f(float(t)) > 0:
            kn[t] = kn.get(t, 0.0) + d
    a0, b0 = segs[0]
    if not (a0 < 0 or (a0 == 0 and b0 > 0)):
        a0, b0 = 0.0, 0.0
    return _PWL(a0, b0, [(t, d) for t, d in kn.items() if d != 0.0])


def _pwl_form(W1, B1, W2, B2, W3, B3, W4, B4):
    """-> (A, B, [(d, t), ...]) with knees restricted to [-RFOLD, RFOLD]."""
    x_id = _PWL(1.0, 0.0, [])
    h1 = [_relu_pwl(_lincomb([x_id], [W1[i]], B1[i])) for i in range(4)]
    h2 = [_relu_pwl(_lincomb(h1, W2[j], B2[j])) for j in range(8)]
    h3 = [_relu_pwl(_lincomb(h2, W3[k], B3[k])) for k in range(4)]
    out = _lincomb(h3, W4, B4)
    A, Bc = out.a0, out.b0
    terms = []
    for t, d in out.knees:
        if t < -RFOLD:
            A += d
            Bc += -d * t
        elif t <= RFOLD:
            terms.append((d, t))
    return A, Bc, terms


def _group_quads(pos, neg):
    """Partition populations into NQ quads minimizing sum_q max(pos)+max(neg).

    Greedy seed (sorted by total terms) + pairwise-swap local search.
    """
    n = len(pos)
    order = sorted(range(n), key=lambda i: -(pos[i] + neg[i]))
    quads = [order[PPT * q:PPT * q + PPT] for q in range(NQ)]

    def qcost(quad):
        return max(pos[i] for i in quad) + max(neg[i] for i in quad)

    cost = [qcost(qd) for qd in quads]
    improved = True
    while improved:
        improved = False
        for qa in range(NQ):
            for qb in range(qa + 1, NQ):
                for ia in range(PPT):
                    for ib in range(PPT):
                        a, b = quads[qa][ia], quads[qb][ib]
                        quads[qa][ia], quads[qb][ib] = b, a
                        ca, cb = qcost(quads[qa]), qcost(quads[qb])
                        if ca + cb < cost[qa] + cost[qb]:
                            cost[qa], cost[qb] = ca, cb
                            improved = True
                        else:
                            quads[qa][ia], quads[qb][ib] = a, b
    return quads


# ---------------------------------------------------------------------------
# Device program
# ---------------------------------------------------------------------------

def _build_program(nadd, nsub):
    import concourse.bacc as bacc
    import concourse.mybir as mybir
    from concourse.tile import TileContext

    f32 = mybir.dt.float32
    NK = sum(nadd) + sum(nsub)
    TC = 2 * NK + 2 * NQ  # scale cols, bias cols, A col/quad, B col/quad

    nc = bacc.Bacc("TRN2", target_bir_lowering=False, debug=False,
                   num_devices=NCORES)
    xs = nc.dram_tensor("xs", [NP, SHARD], f32, kind="ExternalInput")
    tab = nc.dram_tensor("tab", [128, TC], f32, kind="ExternalInput")
    ys = nc.dram_tensor("ys", [NP, SHARD], f32, kind="ExternalOutput")

    with TileContext(nc) as tc:
        with tc.tile_pool(name="consts", bufs=1) as cpool, \
             tc.tile_pool(name="xin", bufs=3) as xpool, \
             tc.tile_pool(name="acc", bufs=3) as apool, \
             tc.tile_pool(name="tmp", bufs=6) as tpool:
            tabt = cpool.tile([128, TC], f32)
            nc.sync.dma_start(tabt[:], tab[:, :])
            scratch = cpool.tile([128, 1], f32)
            # absorb the table-DMA wait on ScalarE once, up front
            nc.scalar.activation(scratch[:], tabt[:, 0:1],
                                 mybir.ActivationFunctionType.Copy)

            col = 0
            for q in range(NQ):
                xt = xpool.tile([128, FREE], f32)
                src = xs[PPT * q:PPT * (q + 1), :].rearrange(
                    "i (l f) -> (i l) f", l=LANES)
                nc.sync.dma_start(xt[:], src)

                at = apool.tile([128, FREE], f32)
                nc.vector.tensor_scalar(
                    at[:], xt[:],
                    tabt[:, 2 * NK + q:2 * NK + q + 1],
                    tabt[:, 2 * NK + NQ + q:2 * NK + NQ + q + 1],
                    mybir.AluOpType.mult, mybir.AluOpType.add)

                for j in range(nadd[q] + nsub[q]):
                    op = (mybir.AluOpType.add if j < nadd[q]
                          else mybir.AluOpType.subtract)
                    tt = tpool.tile([128, FREE], f32)
                    nc.scalar.activation(
                        tt[:], xt[:], mybir.ActivationFunctionType.Relu,
                        bias=tabt[:, NK + col:NK + col + 1],
                        scale=tabt[:, col:col + 1])
                    nc.vector.tensor_tensor(at[:], at[:], tt[:], op)
                    col += 1

                dst = ys[PPT * q:PPT * (q + 1), :].rearrange(
                    "i (l f) -> (i l) f", l=LANES)
                nc.sync.dma_start(dst, at[:])

    nc.compile()
    return nc


# ---------------------------------------------------------------------------
# Entry point
# ---------------------------------------------------------------------------

def kernel(X, lin1, lin2, lin3, lin4, b1, b2, b3, b4):
    global LAST_EXEC_NS, LAST_RESULTS

    X = np.ascontiguousarray(np.asarray(X, dtype=np.float32))

    # 1. exact PWL form per population (float64 host math on tiny weights)
    forms = []
    for l in range(NP):
        forms.append(_pwl_form(
            np.asarray(lin1, np.float64)[l, :, 0],
            np.asarray(b1, np.float64)[l, :, 0],
            np.asarray(lin2, np.float64)[l],
            np.asarray(b2, np.float64)[l, :, 0],
            np.asarray(lin3, np.float64)[l],
            np.asarray(b3, np.float64)[l, :, 0],
            np.asarray(lin4, np.float64)[l, 0, :],
            float(np.asarray(b4, np.float64)[l, 0, 0])))

    pos = [sum(1 for d, _ in t if d > 0) for _, _, t in forms]
    neg = [sum(1 for d, _ in t if d <= 0) for _, _, t in forms]

    # 2. schedule: quad grouping + per-quad slot counts
    quads = _group_quads(pos, neg)
    nadd = [max(pos[i] for i in qd) for qd in quads]
    nsub = [max(neg[i] for i in qd) for qd in quads]
    pop_order = [i for qd in quads for i in qd]

    # 3. tables [128, TC]
    NK = sum(nadd) + sum(nsub)
    TC = 2 * NK + 2 * NQ
    tabv = np.zeros((128, TC), dtype=np.float32)
    col = 0
    for q, qd in enumerate(quads):
        ordered_terms = []
        for i in qd:
            _, _, terms = forms[i]
            p = sorted([(d, t) for d, t in terms if d > 0], key=lambda s: s[1])
            m = sorted([(d, t) for d, t in terms if d <= 0], key=lambda s: s[1])
            p += [(0.0, 0.0)] * (nadd[q] - len(p))
            m += [(0.0, 0.0)] * (nsub[q] - len(m))
            ordered_terms.append(p + m)
        for j in range(nadd[q] + nsub[q]):
            for slot in range(PPT):
                d, t = ordered_terms[slot][j]
                a = abs(d)
                rows = slice(slot * LANES, (slot + 1) * LANES)
                tabv[rows, col] = np.float32(a)            # scale
                tabv[rows, NK + col] = np.float32(-a * t)  # bias
            col += 1
    for q, qd in enumerate(quads):
        for slot, i in enumerate(qd):
            A, Bc, _ = forms[i]
            rows = slice(slot * LANES, (slot + 1) * LANES)
            tabv[rows, 2 * NK + q] = np.float32(A)
            tabv[rows, 2 * NK + NQ + q] = np.float32(Bc)

    # 4. program (cached on the schedule signature)
    key = (tuple(nadd), tuple(nsub))
    if key not in _PROGRAM_CACHE:
        _PROGRAM_CACHE[key] = _build_program(nadd, nsub)
    nc = _PROGRAM_CACHE[key]

    # 5. shard inputs: batch split across 8 cores, pops reordered by quad
    Xr = X[pop_order, 0, :]                       # [NP, B]
    Xp = np.zeros((NP, NCORES * SHARD), dtype=np.float32)
    Xp[:, :B] = Xr
    tabv = np.ascontiguousarray(tabv)
    in_maps = [
        {"xs": np.ascontiguousarray(Xp[:, c * SHARD:(c + 1) * SHARD]),
         "tab": tabv}
        for c in range(NCORES)
    ]

    # 6. run on the 8 NeuronCores
    from concourse.bass_utils import run_bass_kernel_spmd
    trace = os.environ.get("K_TRACE", "") == "1"
    res = run_bass_kernel_spmd(nc, in_maps, core_ids=list(range(NCORES)),
                               trace=trace)
    LAST_EXEC_NS = res.exec_time_ns
    LAST_RESULTS = res

    # 7. gather + unshard
    Yr = np.concatenate([res.results[c]["ys"] for c in range(NCORES)],
                        axis=1)[:, :B]
    out = np.empty((NP, 1, B), dtype=np.float32)
    out[pop_order, 0, :] = Yr
    return out
